# revision 53
# baseline (speedup 1.0000x reference)
"""MixLinear GEMM kernel for Trainium2 (8 NeuronCores, column-parallel).

Computes, for full inputs:
    inputs = x.reshape(-1, 4096)
    act_outliers = inputs[:, ind]
    inputs_z = inputs with ind-columns zeroed
    x_scale = clamp(rowmax(|inputs_z|)/127, 1e-8)
    q_x = round(inputs_z / x_scale)                  (|q_x| <= 127 by construction)
    y = (q_x @ q_weight.T) * x_scale * scale_col + act_outliers @ weight_cache.T + bias

Sharding: q_weight/scale_col/weight_cache/bias are sharded along out_features
across the 8 cores (column parallel); x is replicated. Each core produces its
(512, 1376) output shard; the host concatenates.

Kernel design:
- The outlier GEMM is folded into the main GEMM on the host: the transposed
  weight matrix wT[k, o] gets its ind-rows REPLACED by weight_cache[:, j]/sc
  (summed over duplicate indices). The device quantizes the UNMASKED x, so
  q'[k in ind] = round(x_ind/xs) and the single GEMM produces
  y_int + outliers/(xs*sc) in one pass. absmax is computed over MASKED x
  (mask-multiply on DVE, then an absolute-max reduce).
- Weights are pre-transposed/packed on the host to [128, KT, OSH] f16 and
  stay resident in SBUF (~88 KB/partition); no on-device weight transposes.
- x is host-cast to bf16 (halves DMA and doubles DVE throughput; measured
  end-to-end error 8.5e-3 vs the 2e-2 budget).
- Activations are quantized in natural layout (ACT engine applies
  x*recip + 1536; the fp16 write rounds to integer), transposed via the
  XBAR (SP-issued DMA transpose), then fixed up (-1536) in place on ACT.
- Main GEMM: per m-tile, 3 output chunks (512/512/352 wide), 32 matmuls of
  128-contraction accumulating in one PSUM bank each. The matmul stream is
  the ONLY Tensor-engine work, keeping the HAM clock-gate warm (full
  2.4 GHz; ~216 ns per 512-wide matmul).
- Software pipeline: quantize chain runs 2 (rep, mt)-steps ahead of the
  matmuls; epilogues (DVE (psum*xs)*sc + bias, Pool-issued y stores) lag 2
  further steps so PSUM drains are always ready at their queue head.
  Engine-queue assignment keeps chain stages (DVE/ACT/SP) separate from
  drain-dependent work - any mixing serializes the next chain behind the
  previous GEMM's completion.
"""

import sys

import numpy as np

sys.path.insert(0, "/opt/trn_rl_repo")

import concourse.bass as bass  # noqa: E402
import concourse.mybir as mybir  # noqa: E402
import concourse.tile as tile  # noqa: E402
from concourse import bacc  # noqa: E402

N_CORES = 8
M = 512  # 8*64 rows
K = 4096  # in_features
OUT = 11008  # out_features
OSH = OUT // N_CORES  # 1376 per-core shard
FP = 256  # outlier columns
KT = K // 128  # 32 k-tiles
MT = M // 128  # 4 m-tiles
MAGIC = 1536.0  # fp16 spacing is 1.0 in [1024, 2048): forces round-to-int
XH = 2048  # x processed in half-rows (SBUF economy)
CHUNKS = [(0, 512), (512, 512), (1024, 352)]  # (o0, cw) output chunks
WG = 8  # k-tile groups per weight-load DMA

f32 = mybir.dt.float32
f16 = mybir.dt.float16
bf16 = mybir.dt.bfloat16
Alu = mybir.AluOpType
Act = mybir.ActivationFunctionType


def build_program(nrep=1, debug_dump=False):
    """Build the kernel program. nrep>1 emits the whole body nrep times
    (same inputs, same outputs) - used only to measure steady-state HW time
    as (t(nrep) - t(1)) / (nrep - 1)."""
    nc = bacc.Bacc(
        "TRN2", target_bir_lowering=False, debug=False, num_devices=N_CORES
    )

    x_d = nc.dram_tensor("x_in", [M, K], bf16, kind="ExternalInput").ap()
    w_d = nc.dram_tensor("w_in", [128, KT * OSH], f16, kind="ExternalInput").ap()
    mask_d = nc.dram_tensor("mask_in", [1, K], bf16, kind="ExternalInput").ap()
    sc_d = nc.dram_tensor("sc_in", [1, OSH], f32, kind="ExternalInput").ap()
    bias_d = nc.dram_tensor("bias_in", [1, OSH], f32, kind="ExternalInput").ap()
    y_d = nc.dram_tensor("y_out", [M, OSH], f16, kind="ExternalOutput").ap()
    dbg = {}
    if debug_dump:
        for nm, shape, dt in [
            ("dbg_scales", [128, 3 * MT], f32),
            ("dbg_q0", [128, KT * 128], f16),
            ("dbg_q3", [128, KT * 128], f16),
            ("dbg_w0", [128, OSH], f16),
            ("dbg_w31", [128, OSH], f16),
        ]:
            dbg[nm] = nc.dram_tensor(nm, shape, dt, kind="ExternalOutput").ap()

    with tile.TileContext(nc) as tc:
        with (
            tc.tile_pool(name="persist", bufs=1) as persist,
            tc.tile_pool(name="xpool", bufs=8) as xpool,
            tc.tile_pool(name="qnpool", bufs=6) as qnpool,
            tc.tile_pool(name="ypool", bufs=4) as ypool,
            tc.tile_pool(name="psmain", bufs=8, space="PSUM") as psmain,
        ):
            # ---------- persistent tiles ----------
            w_sb = persist.tile([128, KT, OSH], f16)  # resident weights^T
            mask_bc = persist.tile([128, K], bf16)
            sc_bc = persist.tile([128, OSH], f32)
            bias_bc = persist.tile([128, OSH], f32)
            # two tiles per m-tile (one per k-half) so the first half's
            # matmuls can start while the second half is still transposing
            q_tiles = [
                [
                    persist.tile(
                        [128, KT // 2, 128], f16, tag=f"qT{mt}h{h}",
                        name=f"qT{mt}h{h}",
                    )
                    for h in range(2)
                ]
                for mt in range(MT)
            ]
            # bf16 absmax is exact: the max IS one of the bf16 inputs
            am_all = persist.tile([128, MT], bf16)
            am_h = persist.tile([128, 2 * MT], bf16)
            # xs is read by the (lag-4) epilogue of the same mt one rep
            # earlier; rep-parity split avoids that write-after-read coupling
            xs_all = persist.tile([128, 2 * MT], f32)
            recip_all = persist.tile([128, MT], f32)
            # masked-x scratch; bf16 is exact (mask is 0/1, x is bf16)
            xz_scr = persist.tile([128, XH], bf16)

            # ---------- setup ----------
            # resident weights: 4 big HWDGE loads, f16, no transpose needed
            for g in range(KT // WG):
                nc.sync.dma_start(
                    out=w_sb[:, g * WG : (g + 1) * WG, :],
                    in_=w_d[:, g * WG * OSH : (g + 1) * WG * OSH],
                )
            # broadcasts across partitions: DRAM AP with partition-step 0
            nc.gpsimd.dma_start(
                out=mask_bc,
                in_=bass.AP(mask_d.tensor, mask_d.offset, [[0, 128], [1, K]]),
            )
            nc.gpsimd.dma_start(
                out=sc_bc,
                in_=bass.AP(sc_d.tensor, sc_d.offset, [[0, 128], [1, OSH]]),
            )
            nc.gpsimd.dma_start(
                out=bias_bc,
                in_=bass.AP(bias_d.tensor, bias_d.offset, [[0, 128], [1, OSH]]),
            )
            def load_x(rep, mt):
                """x loads, head of the SP HWDGE queue each iteration."""
                ms = slice(mt * 128, (mt + 1) * 128)
                x_hs = []
                for h in range(2):
                    x_h = xpool.tile(
                        [128, XH], bf16, tag="x", name=f"x_{rep}_{mt}_{h}"
                    )
                    nc.sync.dma_start(
                        out=x_h, in_=x_d[ms, h * XH : (h + 1) * XH]
                    )
                    x_hs.append(x_h)
                return x_hs

            def phase1(rep, mt, x_hs):
                """masked absmax -> quantize -> XBAR transpose."""
                for h in range(2):
                    nc.vector.tensor_tensor(
                        out=xz_scr,
                        in0=x_hs[h],
                        in1=mask_bc[:, h * XH : (h + 1) * XH],
                        op=Alu.mult,
                    )
                    nc.vector.tensor_reduce(
                        out=am_h[:, 2 * mt + h : 2 * mt + h + 1],
                        in_=xz_scr,
                        axis=mybir.AxisListType.X,
                        op=Alu.max,
                        apply_absolute_value=True,
                    )
                nc.vector.tensor_reduce(
                    out=am_all[:, mt : mt + 1],
                    in_=am_h[:, 2 * mt : 2 * mt + 2],
                    axis=mybir.AxisListType.X,
                    op=Alu.max,
                    apply_absolute_value=False,
                )
                # xs = max(absmax/127, 1e-8); recip = 1/xs
                xcol = (rep % 2) * MT + mt
                nc.vector.tensor_scalar(
                    xs_all[:, xcol : xcol + 1],
                    am_all[:, mt : mt + 1],
                    1.0 / 127.0,
                    1e-8,
                    Alu.mult,
                    Alu.max,
                )
                nc.vector.reciprocal(
                    out=recip_all[:, mt : mt + 1], in_=xs_all[:, xcol : xcol + 1]
                )
                # order matters on the ACT queue: both quantizes first, then
                # transposes, then fixups - a fixup between the quantizes
                # serializes the whole chain through ACT+XBAR.
                qns = []
                for h in range(2):
                    # q_off = x*recip + 1536 -> fp16 write rounds to int (RNE)
                    qn = qnpool.tile(
                        [128, XH], f16, tag="qn", name=f"qn_{rep}_{mt}_{h}"
                    )
                    nc.scalar.activation(
                        out=qn,
                        in_=x_hs[h],
                        func=Act.Copy,
                        bias=MAGIC,
                        scale=recip_all[:, mt : mt + 1],
                    )
                    qns.append(qn)
                for h in range(2):
                    # XBAR transpose into the k-partition layout.
                    # NOTE: must be issued from the SP sequencer -
                    # ACT-issued xbar transposes corrupt data on HW.
                    nc.sync.dma_start(
                        out=q_tiles[mt][h][:, :, :],
                        in_=qns[h],
                        transpose=True,
                    )
                for h in range(2):
                    # undo the rounding bias in place: q = q_off - 1536.
                    # On ACT (not DVE): keeps DVE's queue head-of-chain only,
                    # so successive quantize chains overlap instead of
                    # serializing behind this XBAR-gated op.
                    q_th = q_tiles[mt][h]
                    nc.scalar.activation(
                        out=q_th[:, :, :],
                        in_=q_th[:, :, :],
                        func=Act.Copy,
                        bias=-MAGIC,
                        scale=1.0,
                    )

            def phase2_mm(rep, mt):
                """main GEMM over output chunks; returns held psum tiles."""
                # all chunks' first-half (h0) matmuls run before any
                # second-half matmul: ~10us of work hides the h1
                # transpose+fixup tail instead of ~3.4us
                pss = [
                    psmain.tile(
                        [128, 512], f32, tag="ps", name=f"ps_{rep}_{mt}_{c}"
                    )
                    for c in range(len(CHUNKS))
                ]
                for h in range(2):
                    q_th = q_tiles[mt][h]
                    for (o0, cw), ps in zip(CHUNKS, pss):
                        for kl in range(KT // 2):
                            kk = h * (KT // 2) + kl
                            nc.tensor.matmul(
                                ps[:, :cw],
                                lhsT=q_th[:, kl, :],
                                rhs=w_sb[:, kk, o0 : o0 + cw],
                                start=(kk == 0),
                                stop=(kk == KT - 1),
                            )
                return pss

            def epilogue(rep, mt, pss):
                ms = slice(mt * 128, (mt + 1) * 128)
                xcol = (rep % 2) * MT + mt
                for (o0, cw), ps in zip(CHUNKS, pss):
                    ysb = ypool.tile([128, 512], f16, tag="ysb")
                    nc.vector.scalar_tensor_tensor(
                        out=ysb[:, :cw],
                        in0=ps[:, :cw],
                        scalar=xs_all[:, xcol : xcol + 1],
                        in1=sc_bc[:, o0 : o0 + cw],
                        op0=Alu.mult,
                        op1=Alu.mult,
                    )
                    # on DVE, not Pool: a Pool-hosted bias blocks the x-load
                    # descgen queued behind it until the DVE catches up
                    nc.vector.tensor_tensor(
                        out=ysb[:, :cw],
                        in0=ysb[:, :cw],
                        in1=bias_bc[:, o0 : o0 + cw],
                        op=Alu.add,
                    )
                    nc.gpsimd.dma_start(
                        out=y_d[ms, o0 : o0 + cw], in_=ysb[:, :cw]
                    )

            # Software pipeline. Per-iteration emission order matters:
            # x-loads first (head of the SWDGE queue), matmuls lagging the
            # quantize chain by 2 steps (the chain tail is fully hidden
            # under ~37us of matmuls), epilogues 2 further behind (their
            # psums are guaranteed drained-ready, so they never gate the
            # following quantize chain on DVE), then this step's chain.
            steps = [(rep, mt) for rep in range(nrep) for mt in range(MT)]
            xq, psq = [], []
            for i in range(len(steps) + 4):
                if i < len(steps):
                    xq.append(load_x(*steps[i]))
                if 2 <= i < len(steps) + 2:
                    psq.append(phase2_mm(*steps[i - 2]))
                if i >= 4:
                    epilogue(*steps[i - 4], psq[i - 4])
                if i < len(steps):
                    phase1(*steps[i], xq[i])

            if debug_dump:
                    nc.sync.dma_start(
                        out=dbg["dbg_scales"][:, MT : 2 * MT], in_=xs_all[:, 0:MT]
                    )
                    nc.sync.dma_start(
                        out=dbg["dbg_scales"][:, 2 * MT : 3 * MT], in_=recip_all
                    )
                    for h in range(2):
                        hw_ = KT // 2 * 128
                        nc.sync.dma_start(
                            out=dbg["dbg_q0"][:, h * hw_ : (h + 1) * hw_],
                            in_=q_tiles[0][h][:, :, :],
                        )
                        nc.sync.dma_start(
                            out=dbg["dbg_q3"][:, h * hw_ : (h + 1) * hw_],
                            in_=q_tiles[3][h][:, :, :],
                        )
                    nc.sync.dma_start(out=dbg["dbg_w0"], in_=w_sb[:, 0, :])
                    nc.sync.dma_start(out=dbg["dbg_w31"], in_=w_sb[:, 31, :])

    nc.compile()
    return nc


_NC_CACHE = None


def get_program():
    global _NC_CACHE
    if _NC_CACHE is None:
        _NC_CACHE = build_program()
    return _NC_CACHE


def make_in_maps(x, q_weight, scale_col, weight_cache, ind, bias):
    import ml_dtypes

    x2 = np.ascontiguousarray(
        np.asarray(x, dtype=np.float32).reshape(M, K).astype(ml_dtypes.bfloat16)
    )
    q_weight = np.asarray(q_weight, dtype=np.int32)
    scale_col = np.asarray(scale_col, dtype=np.float32).reshape(OUT)
    weight_cache = np.asarray(weight_cache, dtype=np.float32)
    ind_np = np.asarray(ind, dtype=np.int32).reshape(FP)
    bias_np = np.asarray(bias, dtype=np.float32).reshape(OUT)

    mask = np.ones(K, dtype=np.float32)
    mask[ind_np] = 0.0
    mask_bf = mask.astype(ml_dtypes.bfloat16).reshape(1, K)

    in_maps = []
    for c in range(N_CORES):
        sl = slice(c * OSH, (c + 1) * OSH)
        sc_sh = scale_col[sl]
        w_sh = q_weight[sl]
        cache_sh = weight_cache[sl]
        # Folded transposed weights: wT[k, o] = W[o, k], with ind-rows
        # replaced by sum-over-duplicates of cache[:, j]/sc (those W entries
        # only ever multiply the zeroed activation columns in the reference).
        wT = w_sh.T.astype(np.float32)  # (K, OSH)
        acc = np.zeros((K, OSH), dtype=np.float32)
        np.add.at(acc, ind_np, (cache_sh / sc_sh[:, None]).T)
        wT[mask == 0.0] = acc[mask == 0.0]
        w_pack = np.ascontiguousarray(
            wT.reshape(KT, 128, OSH).transpose(1, 0, 2).reshape(128, KT * OSH)
        ).astype(np.float16)
        in_maps.append(
            {
                "x_in": x2,
                "w_in": w_pack,
                "mask_in": mask_bf,
                "sc_in": np.ascontiguousarray(sc_sh.reshape(1, OSH)),
                "bias_in": np.ascontiguousarray(bias_np[sl].reshape(1, OSH)),
            }
        )
    return in_maps


def kernel(x, q_weight, scale_col, weight_cache, ind, bias):
    from concourse.bass_utils import run_bass_kernel_spmd

    nc = get_program()
    in_maps = make_in_maps(x, q_weight, scale_col, weight_cache, ind, bias)
    res = run_bass_kernel_spmd(nc, in_maps, core_ids=list(range(N_CORES)))
    shards = [res.results[c]["y_out"] for c in range(N_CORES)]
    y = np.concatenate(shards, axis=1)
    return y.reshape(8, 64, OUT).astype(np.float32)
